# revision 16
# baseline (speedup 1.0000x reference)
"""Adaptive thresholding (11x11 box mean, BORDER_REPLICATE, THRESH_BINARY_INV)
on 8 TRN2 NeuronCores, data-parallel over the batch dim.

V4 design, per 512x512 image (fp16 data path), 4 row-blocks of 128:
  - x DMA'd as fp16 into ximg [128, 4, 533]: per segment, cols 0..10 zeros,
    11..15 left margin, 16..527 x rows, 528..532 right margin.
  - ONE DVE sliding-window scan over the flattened [128, 2121] view:
    state = (xp[t] + state) - xp[t-11]; the 11-col zero head between
    segments self-drains the window state, so segments stay independent.
    Output (fp16) holds the horizontal 11-tap sums W11 per segment.
  - PE per block (all fp16, N=512, one PSUM bank):
      main band  BM^T @ W11_seg          (K=128)
      identity   (-121*I)^T @ x_seg      (K=128)
      halo prev  BHP^T @ W11_prevseg[96:128]  (K=32, tile_position (96,0))
      halo next  BHN^T @ W11_nextseg[0:32]    (K=32, tile_position (0,0))
    PSUM = 121*(mean - x).
  - ACT: Sign(PSUM - 242) -> bf16 {-1,0,+1}.
Host: out = (sign >= 0) * 255  (inclusive compare matches x <= mean-2).
"""
import sys
sys.path.insert(0, '/opt/trn_rl_repo')
import numpy as np
import concourse.bass as bass
import concourse.tile as tile
from concourse import bacc, mybir
from concourse.bass_utils import run_bass_kernel_spmd
F32 = mybir.dt.float32
F16 = mybir.dt.float16
BF16 = mybir.dt.bfloat16

N_CORES = 8
BATCH, H, W = 128, 512, 512
IMGS_PER_CORE = BATCH // N_CORES      # 16
ROWS_PER_CORE = IMGS_PER_CORE * H     # 8192
BLK = 128
NBLK = H // BLK                       # 4
K = 11
PAD = K // 2                          # 5
ZH = K                                # zero head width
WT = ZH + PAD + W + PAD               # 533 segment width
XP0 = ZH                              # xp offset within segment (11)
X0 = ZH + PAD                         # x offset within segment (16)
FLAT = NBLK * WT                      # 2132
SCLEN = FLAT - ZH                     # 2121 scan steps
KH = 32                               # halo row-group size


def _band_matrices(dtype=np.float16):
    r = np.arange(BLK)
    bm_mid = (np.abs(r[:, None] - r[None, :]) <= PAD).astype(dtype)
    bm_top = bm_mid.copy()
    for rr in range(PAD):
        bm_top[0, rr] += dtype(PAD - rr)
    bm_bot = bm_mid.copy()
    for rr in range(BLK - PAD, BLK):
        bm_bot[BLK - 1, rr] += dtype(rr - (BLK - PAD - 1))
    # halo prev: weight rows are prev-segment partitions 96..127 (rel 0..31);
    # partition 96+p is image row (seg base - 32 + p); nonzero for p>=27:
    # row k = -32+p affects output r iff |r - k| <= 5 -> r <= p - 27.
    bhp = np.zeros((BLK, BLK), dtype=dtype)
    for p in range(BLK - PAD, BLK):
        bhp[p, 0:p - (BLK - PAD) + 1] = 1.0
    # halo next: partitions 0..31 of next segment = image rows 128+p;
    # affects r iff r >= 123+p, for p in 0..4.
    bhn = np.zeros((BLK, BLK), dtype=dtype)
    for p in range(PAD):
        bhn[p, BLK - PAD + p:BLK] = 1.0
    idn = (-121.0 * np.eye(BLK)).astype(dtype)
    return {"bm_top": bm_top, "bm_mid": bm_mid, "bm_bot": bm_bot,
            "bhp": bhp, "bhn": bhn, "idn": idn}


def _build():
    nc = bacc.Bacc(None, target_bir_lowering=False, debug=False)
    x_d = nc.declare_dram_parameter("x", [ROWS_PER_CORE, W], F16, isOutput=False)
    shapes = {"bm_top": [BLK, BLK], "bm_mid": [BLK, BLK], "bm_bot": [BLK, BLK],
              "bhp": [BLK, BLK], "bhn": [BLK, BLK], "idn": [BLK, BLK]}
    consts = {nm: nc.declare_dram_parameter(nm, sh, F16, isOutput=False)
              for nm, sh in shapes.items()}
    out_d = nc.declare_dram_parameter("out", [ROWS_PER_CORE, W], BF16, isOutput=True)
    xv = x_d[:].rearrange("(i p q) c -> i p q c", p=NBLK, q=BLK)   # [16,4,128,512]
    ov = out_d[:].rearrange("(i p q) c -> i p q c", p=NBLK, q=BLK)

    with tile.TileContext(nc) as tc:
        with (
            tc.tile_pool(name="cpool", bufs=1) as cpool,
            tc.tile_pool(name="xin", bufs=4) as x_pool,
            tc.tile_pool(name="scr", bufs=4) as s_pool,
            tc.tile_pool(name="outp", bufs=3) as o_pool,
            tc.tile_pool(name="psum", bufs=8, space=bass.MemorySpace.PSUM) as ps_pool,
        ):
            ct = {}
            for nm, d in consts.items():
                t = cpool.tile(list(d.shape), F16, tag=nm)
                nc.sync.dma_start(t[:], d[:])
                ct[nm] = t
            bias_t = cpool.tile([BLK, 1], F32, tag="bias")
            nc.vector.memset(bias_t[:], -242.0)

            imgs = {}  # i -> (ximg, scr_img, oimg)

            def front_img(i):
                ximg = x_pool.tile([BLK, NBLK, WT], F16, tag="ximg")
                nc.sync.dma_start(
                    ximg[:, :, X0:X0 + W],
                    xv[i].rearrange("p q c -> q p c"))
                nc.gpsimd.memset(ximg[:, :, 0:ZH], 0.0)
                nc.gpsimd.tensor_copy(
                    ximg[:, :, XP0:X0],
                    ximg[:, :, X0:X0 + 1].to_broadcast((BLK, NBLK, PAD)))
                nc.gpsimd.tensor_copy(
                    ximg[:, :, X0 + W:WT],
                    ximg[:, :, X0 + W - 1:X0 + W].to_broadcast((BLK, NBLK, PAD)))
                flat = ximg[:].rearrange("q p c -> q (p c)")
                s = s_pool.tile([BLK, SCLEN], F16, tag="scr")
                nc.vector.tensor_tensor_scan(
                    s[:], flat[:, ZH:FLAT], flat[:, 0:SCLEN], 0.0,
                    op0=mybir.AluOpType.add, op1=mybir.AluOpType.subtract)
                oimg = o_pool.tile([BLK, NBLK, W], BF16, tag="oimg")
                imgs[i] = (ximg, s, oimg)

            def back_img(i):
                ximg, s, oimg = imgs.pop(i)
                # matmuls grouped by weight matrix across the 4 psum banks so
                # walrus ldw-opt amortizes LDWEIGHTS; per-bank group order:
                # bm (start) -> idn -> bhn -> bhp (last touch carries stop).
                pss = [ps_pool.tile([BLK, W], F32, tag="ps", name=f"ps_{i}_{j}")
                       for j in range(NBLK)]

                def segof(pos):
                    return pos * WT + (K - 1)

                for pos in range(NBLK):
                    sfx = "top" if pos == 0 else ("bot" if pos == NBLK - 1 else "mid")
                    nc.tensor.matmul(pss[pos][:], ct["bm_" + sfx][:],
                                     s[:, segof(pos):segof(pos) + W],
                                     start=True, stop=False)
                for pos in range(NBLK):
                    nc.tensor.matmul(pss[pos][:], ct["idn"][:],
                                     ximg[:, pos, X0:X0 + W],
                                     start=False, stop=False)
                for pos in range(NBLK - 1):
                    # bank 0's accumulation ends here (it gets no bhp term)
                    nc.tensor.matmul(pss[pos][:], ct["bhn"][:],
                                     s[:, segof(pos + 1):segof(pos + 1) + W],
                                     start=False, stop=(pos == 0))
                for pos in range(1, NBLK):
                    nc.tensor.matmul(pss[pos][:], ct["bhp"][:],
                                     s[:, segof(pos - 1):segof(pos - 1) + W],
                                     start=False, stop=True)
                # bank 0's last touch is its bhn (pos==NBLK-2 above): fix stops
                for pos in range(NBLK):
                    nc.scalar.activation(
                        oimg[:, pos, :], pss[pos][:],
                        mybir.ActivationFunctionType.Sign,
                        bias=bias_t[:], scale=1.0)
                nc.sync.dma_start(ov[i].rearrange("p q c -> q p c"), oimg[:])

            front_img(0)
            front_img(1)
            front_img(2)
            for i in range(IMGS_PER_CORE):
                back_img(i)
                if i + 3 < IMGS_PER_CORE:
                    front_img(i + 3)
    nc.compile()
    return nc


_NC_CACHE = None


def kernel(x: np.ndarray) -> np.ndarray:
    global _NC_CACHE
    x = np.asarray(x, dtype=np.float32)
    x16 = x.reshape(BATCH, H, W).astype(np.float16)

    consts = _band_matrices()
    if _NC_CACHE is None:
        _NC_CACHE = _build()
    nc = _NC_CACHE

    in_maps = []
    for c in range(N_CORES):
        shard = x16[c * IMGS_PER_CORE:(c + 1) * IMGS_PER_CORE].reshape(
            ROWS_PER_CORE, W)
        m = {"x": np.ascontiguousarray(shard)}
        m.update(consts)
        in_maps.append(m)
    res = run_bass_kernel_spmd(nc, in_maps, core_ids=list(range(N_CORES)))
    out = np.empty((BATCH, H, W), dtype=np.float32)
    for c in range(N_CORES):
        sgn = res.results[c]["out"].astype(np.float32)
        out[c * IMGS_PER_CORE:(c + 1) * IMGS_PER_CORE] = \
            ((sgn >= 0.0) * np.float32(255.0)).reshape(IMGS_PER_CORE, H, W)
    return out.reshape(BATCH, H, W, 1)


# revision 17
# speedup vs baseline: 1.0601x; 1.0601x over previous
"""Adaptive thresholding (11x11 box mean, BORDER_REPLICATE, THRESH_BINARY_INV)
on 8 TRN2 NeuronCores, data-parallel over the batch dim.

V4 design, per 512x512 image (fp16 data path), 4 row-blocks of 128:
  - x DMA'd as fp16 into ximg [128, 4, 533]: per segment, cols 0..10 zeros,
    11..15 left margin, 16..527 x rows, 528..532 right margin.
  - ONE DVE sliding-window scan over the flattened [128, 2121] view:
    state = (xp[t] + state) - xp[t-11]; the 11-col zero head between
    segments self-drains the window state, so segments stay independent.
    Output (fp16) holds the horizontal 11-tap sums W11 per segment.
  - PE per block (all fp16, N=512, one PSUM bank):
      main band  BM^T @ W11_seg          (K=128)
      identity   (-121*I)^T @ x_seg      (K=128)
      halo prev  BHP^T @ W11_prevseg[96:128]  (K=32, tile_position (96,0))
      halo next  BHN^T @ W11_nextseg[0:32]    (K=32, tile_position (0,0))
    PSUM = 121*(mean - x).
  - ACT: Sign(PSUM - 242) -> bf16 {-1,0,+1}.
Host: out = (sign >= 0) * 255  (inclusive compare matches x <= mean-2).
"""
import sys
sys.path.insert(0, '/opt/trn_rl_repo')
import numpy as np
import concourse.bass as bass
import concourse.tile as tile
from concourse import bacc, mybir
from concourse.bass_utils import run_bass_kernel_spmd
F32 = mybir.dt.float32
F16 = mybir.dt.float16
BF16 = mybir.dt.bfloat16

N_CORES = 8
BATCH, H, W = 128, 512, 512
IMGS_PER_CORE = BATCH // N_CORES      # 16
ROWS_PER_CORE = IMGS_PER_CORE * H     # 8192
BLK = 128
NBLK = H // BLK                       # 4
K = 11
PAD = K // 2                          # 5
ZH = K                                # zero head width
WT = ZH + PAD + W + PAD               # 533 segment width
XP0 = ZH                              # xp offset within segment (11)
X0 = ZH + PAD                         # x offset within segment (16)
FLAT = NBLK * WT                      # 2132
SCLEN = FLAT - ZH                     # 2121 scan steps
KH = 32                               # halo row-group size


def _band_matrices(dtype=np.float16):
    r = np.arange(BLK)
    bm_mid = (np.abs(r[:, None] - r[None, :]) <= PAD).astype(dtype)
    bm_top = bm_mid.copy()
    for rr in range(PAD):
        bm_top[0, rr] += dtype(PAD - rr)
    bm_bot = bm_mid.copy()
    for rr in range(BLK - PAD, BLK):
        bm_bot[BLK - 1, rr] += dtype(rr - (BLK - PAD - 1))
    # halo prev: weight rows are prev-segment partitions 96..127 (rel 0..31);
    # partition 96+p is image row (seg base - 32 + p); nonzero for p>=27:
    # row k = -32+p affects output r iff |r - k| <= 5 -> r <= p - 27.
    bhp = np.zeros((BLK, BLK), dtype=dtype)
    for p in range(BLK - PAD, BLK):
        bhp[p, 0:p - (BLK - PAD) + 1] = 1.0
    # halo next: partitions 0..31 of next segment = image rows 128+p;
    # affects r iff r >= 123+p, for p in 0..4.
    bhn = np.zeros((BLK, BLK), dtype=dtype)
    for p in range(PAD):
        bhn[p, BLK - PAD + p:BLK] = 1.0
    idn = (-121.0 * np.eye(BLK)).astype(dtype)
    return {"bm_top": bm_top, "bm_mid": bm_mid, "bm_bot": bm_bot,
            "bhp": bhp, "bhn": bhn, "idn": idn}


def _build():
    nc = bacc.Bacc(None, target_bir_lowering=False, debug=False)
    x_d = nc.declare_dram_parameter("x", [ROWS_PER_CORE, W], F16, isOutput=False)
    shapes = {"bm_top": [BLK, BLK], "bm_mid": [BLK, BLK], "bm_bot": [BLK, BLK],
              "bhp": [BLK, BLK], "bhn": [BLK, BLK], "idn": [BLK, BLK]}
    consts = {nm: nc.declare_dram_parameter(nm, sh, F16, isOutput=False)
              for nm, sh in shapes.items()}
    out_d = nc.declare_dram_parameter("out", [ROWS_PER_CORE, W], BF16, isOutput=True)
    xv = x_d[:].rearrange("(i p q) c -> i p q c", p=NBLK, q=BLK)   # [16,4,128,512]
    ov = out_d[:].rearrange("(i p q) c -> i p q c", p=NBLK, q=BLK)

    with tile.TileContext(nc) as tc:
        with (
            tc.tile_pool(name="cpool", bufs=1) as cpool,
            tc.tile_pool(name="xin", bufs=4) as x_pool,
            tc.tile_pool(name="scr", bufs=4) as s_pool,
            tc.tile_pool(name="outp", bufs=3) as o_pool,
            tc.tile_pool(name="psum", bufs=8, space=bass.MemorySpace.PSUM) as ps_pool,
        ):
            ct = {}
            for nm, d in consts.items():
                t = cpool.tile(list(d.shape), F16, tag=nm)
                nc.scalar.dma_start(t[:], d[:])
                ct[nm] = t
            bias_t = cpool.tile([BLK, 1], F32, tag="bias")
            nc.vector.memset(bias_t[:], -242.0)

            imgs = {}  # i -> (ximg, scr_img, oimg)

            def front_img(i):
                ximg = x_pool.tile([BLK, NBLK, WT], F16, tag="ximg")
                nc.sync.dma_start(
                    ximg[:, :, X0:X0 + W],
                    xv[i].rearrange("p q c -> q p c"))
                nc.gpsimd.memset(ximg[:, :, 0:ZH], 0.0)
                nc.gpsimd.tensor_copy(
                    ximg[:, :, XP0:X0],
                    ximg[:, :, X0:X0 + 1].to_broadcast((BLK, NBLK, PAD)))
                nc.gpsimd.tensor_copy(
                    ximg[:, :, X0 + W:WT],
                    ximg[:, :, X0 + W - 1:X0 + W].to_broadcast((BLK, NBLK, PAD)))
                flat = ximg[:].rearrange("q p c -> q (p c)")
                s = s_pool.tile([BLK, SCLEN], F16, tag="scr")
                if i == 0:
                    # per-segment scans so the first matmuls start sooner
                    for pos in range(NBLK):
                        o0 = pos * WT
                        nc.vector.tensor_tensor_scan(
                            s[:, o0:o0 + WT - ZH],
                            flat[:, o0 + ZH:o0 + WT], flat[:, o0:o0 + WT - ZH],
                            0.0, op0=mybir.AluOpType.add,
                            op1=mybir.AluOpType.subtract)
                else:
                    nc.vector.tensor_tensor_scan(
                        s[:], flat[:, ZH:FLAT], flat[:, 0:SCLEN], 0.0,
                        op0=mybir.AluOpType.add, op1=mybir.AluOpType.subtract)
                oimg = o_pool.tile([BLK, NBLK, W], BF16, tag="oimg")
                imgs[i] = (ximg, s, oimg)

            def back_img(i):
                ximg, s, oimg = imgs.pop(i)
                # matmuls grouped by weight matrix across the 4 psum banks so
                # walrus ldw-opt amortizes LDWEIGHTS; per-bank group order:
                # bm (start) -> idn -> bhn -> bhp (last touch carries stop).
                pss = [ps_pool.tile([BLK, W], F32, tag="ps", name=f"ps_{i}_{j}")
                       for j in range(NBLK)]

                def segof(pos):
                    return pos * WT + (K - 1)

                for pos in range(NBLK):
                    sfx = "top" if pos == 0 else ("bot" if pos == NBLK - 1 else "mid")
                    nc.tensor.matmul(pss[pos][:], ct["bm_" + sfx][:],
                                     s[:, segof(pos):segof(pos) + W],
                                     start=True, stop=False)
                for pos in range(NBLK):
                    nc.tensor.matmul(pss[pos][:], ct["idn"][:],
                                     ximg[:, pos, X0:X0 + W],
                                     start=False, stop=False)
                for pos in range(NBLK - 1):
                    # bank 0's accumulation ends here (it gets no bhp term)
                    nc.tensor.matmul(pss[pos][:], ct["bhn"][:],
                                     s[:, segof(pos + 1):segof(pos + 1) + W],
                                     start=False, stop=(pos == 0))
                for pos in range(1, NBLK):
                    nc.tensor.matmul(pss[pos][:], ct["bhp"][:],
                                     s[:, segof(pos - 1):segof(pos - 1) + W],
                                     start=False, stop=True)
                # bank 0's last touch is its bhn (pos==NBLK-2 above): fix stops
                for pos in range(NBLK):
                    nc.scalar.activation(
                        oimg[:, pos, :], pss[pos][:],
                        mybir.ActivationFunctionType.Sign,
                        bias=bias_t[:], scale=1.0)
                    if i == IMGS_PER_CORE - 1:
                        nc.sync.dma_start(ov[i, pos], oimg[:, pos, :])
                if i != IMGS_PER_CORE - 1:
                    nc.sync.dma_start(ov[i].rearrange("p q c -> q p c"), oimg[:])

            front_img(0)
            front_img(1)
            front_img(2)
            for i in range(IMGS_PER_CORE):
                back_img(i)
                if i + 3 < IMGS_PER_CORE:
                    front_img(i + 3)
    nc.compile()
    return nc


_NC_CACHE = None


def kernel(x: np.ndarray) -> np.ndarray:
    global _NC_CACHE
    x = np.asarray(x, dtype=np.float32)
    x16 = x.reshape(BATCH, H, W).astype(np.float16)

    consts = _band_matrices()
    if _NC_CACHE is None:
        _NC_CACHE = _build()
    nc = _NC_CACHE

    in_maps = []
    for c in range(N_CORES):
        shard = x16[c * IMGS_PER_CORE:(c + 1) * IMGS_PER_CORE].reshape(
            ROWS_PER_CORE, W)
        m = {"x": np.ascontiguousarray(shard)}
        m.update(consts)
        in_maps.append(m)
    res = run_bass_kernel_spmd(nc, in_maps, core_ids=list(range(N_CORES)))
    out = np.empty((BATCH, H, W), dtype=np.float32)
    for c in range(N_CORES):
        sgn = res.results[c]["out"].astype(np.float32)
        out[c * IMGS_PER_CORE:(c + 1) * IMGS_PER_CORE] = \
            ((sgn >= 0.0) * np.float32(255.0)).reshape(IMGS_PER_CORE, H, W)
    return out.reshape(BATCH, H, W, 1)
